# revision 8
# baseline (speedup 1.0000x reference)
"""Trainium2 Bass kernel for nn_MultiHeadAttention (B=2, S=2048, D=1024, H=16, dk=64).

Sharding: 8 cores = (batch b in {0,1}) x (head group g in {0..3}, 4 heads each).
Key observation: the reference does a RAW reshape (B,H,S,dk) -> (B,S,H*dk)
(mixing head and sequence axes), so output row s' = h*128 + s//16 of X @ WO
depends ONLY on head h.  Core (b,g) therefore produces output rows
[512g, 512(g+1)) of batch b -- a pure concatenation, no collectives.

Per-core pipeline (all matmuls float32r, fp32 PSUM accumulate):
  1. QpT/KpT = W^T @ x^T   -> [heads*dk, S] layouts (host pre-transposes Q,K,V)
     Vp      = x^T.T @ Wv  -> [S, heads*dk] natural layout, augmented with a
               ones column per head (denominator trick).
  2. Per head pair, per s_q block of 512:
     scores^T[s_k, s_q] tiles on PE (2 heads packed via row tile_position),
     exp via ACT (scale=1/8 fused, no max subtraction -- fp32 exp is safe
     for |score*scale| <~ 25), P@V with V_aug -> PSUM [65, 512] where row 64
     is the softmax denominator.
  3. recip (DVE) -> broadcast via K=1 outer-product matmul -> normalize on DVE,
     writing the head/seq-mixed layout directly with a strided AP.
  4. One scatter DMA per (head, partition parity) assembles X^T tiles;
     WO matmul per head; DMA out.
"""

import sys

try:
    import concourse.bass as bass  # noqa: F401
except ImportError:
    sys.path.insert(0, "/opt/trn_rl_repo")

import numpy as np

import concourse.bacc as bacc
import concourse.tile as tile
from concourse import mybir
from concourse.bass_utils import run_bass_kernel_spmd

F32R = mybir.dt.float32r
F32 = mybir.dt.float32

B, S, D, H, DK = 2, 2048, 1024, 16, 64
HEADS_PER_CORE = 4
GROUPS = 4
SCALE = 1.0 / 8.0  # 1/sqrt(dk)
E_BUFS = 18

_cached_nc = None


def build_nc():
    nc = bacc.Bacc(None, target_bir_lowering=False)
    qT = nc.dram_tensor("qT", [D, S], F32R, kind="ExternalInput")
    kT = nc.dram_tensor("kT", [D, S], F32R, kind="ExternalInput")
    vT = nc.dram_tensor("vT", [D, S], F32R, kind="ExternalInput")
    wq = nc.dram_tensor("wq", [D, 256], F32R, kind="ExternalInput")
    wk = nc.dram_tensor("wk", [D, 256], F32R, kind="ExternalInput")
    wv = nc.dram_tensor("wv", [D, 256], F32R, kind="ExternalInput")
    wo = nc.dram_tensor("wo", [D, D], F32R, kind="ExternalInput")
    out = nc.dram_tensor("out", [512, D], F32, kind="ExternalOutput")

    Exp = mybir.ActivationFunctionType.Exp

    with tile.TileContext(nc) as tc, nc.allow_low_precision(
        reason="float32r tiles hold full fp32 bits; PSUM accumulation is fp32"
    ):
        with (
            tc.tile_pool(name="persist", bufs=1) as persist,
            tc.tile_pool(name="hrp", bufs=2) as hrp,
            tc.tile_pool(name="xhp", bufs=3) as xhp,
            tc.tile_pool(name="small", bufs=4) as small,
            tc.tile_pool(name="opool", bufs=2) as opool,
        ):
            wo_sb = persist.tile([128, 8, D], F32R, tag="wo")
            nc.sync.dma_start(out=wo_sb, in_=wo.rearrange("(t p) n -> p t n", p=128))
            qpT = persist.tile([128, 2, S], F32R, tag="qpT")
            kpT = persist.tile([128, 2, S], F32R, tag="kpT")
            vaug = persist.tile([128, 16, 4, 65], F32R, tag="vaug")
            ones_f32 = persist.tile([128, 64], F32, tag="ones_f32")
            nc.vector.memset(ones_f32, 1.0)
            nc.vector.tensor_copy(
                vaug[:, :, :, 64:65], ones_f32[:, 0:1].to_broadcast((128, 16, 4, 1))
            )
            ones = persist.tile([1, 64], F32R, tag="ones")
            nc.vector.tensor_copy(ones, ones_f32[0:1, :])

            # ---------------- Phase A: projections ----------------
            with (
                tc.tile_pool(name="wqkv", bufs=1) as wqkv,
                tc.tile_pool(name="stream", bufs=3) as stream,
                tc.tile_pool(name="pps", bufs=4, space="PSUM") as pps,
            ):
                wq_sb = wqkv.tile([128, 8, 256], F32R, tag="wq")
                wk_sb = wqkv.tile([128, 8, 256], F32R, tag="wk")
                wv_sb = wqkv.tile([128, 8, 256], F32R, tag="wv")
                for w_dram, w_sb in ((wq, wq_sb), (wk, wk_sb), (wv, wv_sb)):
                    nc.sync.dma_start(
                        out=w_sb, in_=w_dram.rearrange("(t p) n -> p t n", p=128)
                    )

                # Q, K projections: out^T form [head-pair*128, s] = W^T @ x^T
                for x_dram, w_sb, outt in ((qT, wq_sb, qpT), (kT, wk_sb, kpT)):
                    for nb in range(4):  # s blocks of 512
                        st = stream.tile([128, 8, 512], F32R, tag="acts")
                        nc.sync.dma_start(
                            out=st,
                            in_=x_dram.rearrange("(t p) s -> p t s", p=128)[
                                :, :, 512 * nb : 512 * (nb + 1)
                            ],
                        )
                        for m in range(2):
                            ps = pps.tile([128, 512], F32, tag="pps")
                            for k in range(8):
                                nc.tensor.matmul(
                                    ps,
                                    w_sb[:, k, 128 * m : 128 * (m + 1)],
                                    st[:, k, :],
                                    start=(k == 0),
                                    stop=(k == 7),
                                )
                            nc.vector.tensor_copy(
                                outt[:, m, 512 * nb : 512 * (nb + 1)], ps
                            )

                # V projection: natural form [s, 4*64] (lhsT = vT tile)
                for nb in range(4):
                    st = stream.tile([128, 8, 512], F32R, tag="acts")
                    nc.sync.dma_start(
                        out=st,
                        in_=vT.rearrange("(t p) s -> p t s", p=128)[
                            :, :, 512 * nb : 512 * (nb + 1)
                        ],
                    )
                    for sti in range(4):
                        stt = 4 * nb + sti
                        ps_full = pps.tile([128, 512], F32, tag="pps", name="vps")
                        ps = ps_full[:, :256]
                        for k in range(8):
                            nc.tensor.matmul(
                                ps,
                                st[:, k, 128 * sti : 128 * (sti + 1)],
                                wv_sb[:, k, :],
                                start=(k == 0),
                                stop=(k == 7),
                            )
                        nc.vector.tensor_copy(
                            vaug[:, stt, :, 0:64],
                            ps.rearrange("p (h c) -> p h c", h=4),
                        )

            # ---------------- Phase B/C: attention + output ----------------
            with (
                tc.tile_pool(name="epool", bufs=E_BUFS) as epool,
                tc.tile_pool(name="ps_sc", bufs=2, space="PSUM") as ps_sc,
                tc.tile_pool(name="ps_pv", bufs=2, space="PSUM") as ps_pv,
                tc.tile_pool(name="ps_bc", bufs=1, space="PSUM") as ps_bc,
                tc.tile_pool(name="ps_wo", bufs=1, space="PSUM") as ps_wo,
            ):
                for hp in range(2):
                    hA, hB = 2 * hp, 2 * hp + 1
                    hr = {
                        h: hrp.tile([64, 2048], F32R, tag="hr", name=f"hr{h}") for h in (hA, hB)
                    }
                    for qb in range(4):
                        pv = {
                            h: ps_pv.tile([65, 512], F32, tag="pv", name=f"pv{h}") for h in (hA, hB)
                        }
                        for kp in range(8):  # pairs of k tiles
                            sc = {
                                h: ps_sc.tile([128, 1024], F32, tag="sc", name=f"sc{h}")
                                for h in (hA, hB)
                            }
                            for half in range(2):
                                kt = 2 * kp + half
                                for i, h in enumerate((hA, hB)):
                                    nc.tensor.matmul(
                                        sc[h][:, 512 * half : 512 * (half + 1)],
                                        kpT[64 * i : 64 * (i + 1), hp,
                                            128 * kt : 128 * (kt + 1)],
                                        qpT[64 * i : 64 * (i + 1), hp,
                                            512 * qb : 512 * (qb + 1)],
                                        start=True,
                                        stop=True,
                                        tile_position=(64 * i, 0),
                                    )
                            e = {}
                            for h in (hA, hB):
                                e[h] = epool.tile([128, 1024], F32R, tag="e", name=f"e{h}")
                                nc.scalar.activation(e[h], sc[h], Exp, scale=SCALE)
                            for half in range(2):
                                kt = 2 * kp + half
                                for h in (hA, hB):
                                    nc.tensor.matmul(
                                        pv[h],
                                        vaug[:, kt, h, :],
                                        e[h][:, 512 * half : 512 * (half + 1)],
                                        start=(kt == 0),
                                        stop=(kt == 15),
                                    )
                        # normalize + scatter-layout write
                        for h in (hA, hB):
                            rc = small.tile([1, 512], F32R, tag="rc")
                            nc.vector.reciprocal(rc, pv[h][64:65, :])
                            bc = ps_bc.tile([64, 512], F32, tag="bc")
                            nc.tensor.matmul(bc, ones, rc, start=True, stop=True)
                            bc_sb = small.tile([64, 512], F32R, tag="bcs", name=f"bcs{h}")
                            nc.vector.tensor_copy(bc_sb, bc)
                            hview = hr[h].rearrange("p (j r) -> p r j", j=16)[
                                :, 32 * qb : 32 * (qb + 1), :
                            ]
                            nc.vector.tensor_mul(hview, pv[h][0:64, :], bc_sb)

                    # scatter DMA into X^T layout + WO matmul per head
                    for h in (hA, hB):
                        xh = xhp.tile([128, 8, 128], F32R, tag="xh")
                        hv = hr[h].rearrange("p (j r) -> p j r", j=16)
                        for par in range(2):
                            nc.sync.dma_start(
                                out=xh[64 * par : 64 * (par + 1)],
                                in_=hv[:, par::2, :],
                            )
                        for n in range(2):
                            wops = ps_wo.tile([128, 512], F32, tag="wo")
                            for t in range(8):
                                nc.tensor.matmul(
                                    wops,
                                    xh[:, t, :],
                                    wo_sb[:, t, 512 * n : 512 * (n + 1)],
                                    start=(t == 0),
                                    stop=(t == 7),
                                )
                            ot = opool.tile([128, 512], F32, tag="o")
                            nc.vector.tensor_copy(ot, wops)
                            nc.sync.dma_start(
                                out=out[128 * h : 128 * (h + 1),
                                        512 * n : 512 * (n + 1)],
                                in_=ot,
                            )

    nc.finalize()
    return nc


def make_in_maps(Q, K, V, WQ, WK, WV, WO):
    in_maps = []
    wo_full = np.ascontiguousarray(WO.astype(np.float32))
    for b in range(B):
        qTb = np.ascontiguousarray(Q[b].T.astype(np.float32))
        kTb = np.ascontiguousarray(K[b].T.astype(np.float32))
        vTb = np.ascontiguousarray(V[b].T.astype(np.float32))
        for g in range(GROUPS):
            hs = slice(4 * g, 4 * g + 4)
            # [4, D, dk] -> [D, 4*dk]
            wqc = np.ascontiguousarray(
                WQ[hs].transpose(1, 0, 2).reshape(D, 256).astype(np.float32)
            )
            wkc = np.ascontiguousarray(
                WK[hs].transpose(1, 0, 2).reshape(D, 256).astype(np.float32)
            )
            wvc = np.ascontiguousarray(
                WV[hs].transpose(1, 0, 2).reshape(D, 256).astype(np.float32)
            )
            in_maps.append(
                {"qT": qTb, "kT": kTb, "vT": vTb,
                 "wq": wqc, "wk": wkc, "wv": wvc, "wo": wo_full}
            )
    return in_maps


def run(inputs, **run_kwargs):
    global _cached_nc
    if _cached_nc is None:
        _cached_nc = build_nc()
    in_maps = make_in_maps(**inputs)
    res = run_bass_kernel_spmd(
        _cached_nc, in_maps, core_ids=list(range(8)), **run_kwargs
    )
    full = np.zeros((B, S, D), np.float32)
    for b in range(B):
        for g in range(GROUPS):
            full[b, 512 * g : 512 * (g + 1), :] = res.results[4 * b + g]["out"]
    return full, res


def kernel(**inputs):
    full, _ = run(inputs)
    return full


if __name__ == "__main__":
    rng = np.random.default_rng(0)
    inputs = {
        "Q": rng.standard_normal((B, S, D)).astype(np.float32),
        "K": rng.standard_normal((B, S, D)).astype(np.float32),
        "V": rng.standard_normal((B, S, D)).astype(np.float32),
        "WQ": (rng.uniform(-0.1, 0.1, (H, D, DK))).astype(np.float32),
        "WK": (rng.uniform(-0.1, 0.1, (H, D, DK))).astype(np.float32),
        "WV": (rng.uniform(-0.1, 0.1, (H, D, DK))).astype(np.float32),
        "WO": (rng.uniform(-0.1, 0.1, (H * DK, D))).astype(np.float32),
    }
    out = kernel(**inputs)
    print("kernel out", out.shape, out.dtype, float(np.abs(out).max()))
